# revision 11
# baseline (speedup 1.0000x reference)
"""Trainium2 kernel for nn_PennyLaneQuantumClassifier.

Math: the quantum circuit is linear in the state vector, and the state is
amplitude-encoded from only N_INPUTS=10 real amplitudes.  Hence the PauliZ
expectation collapses to a quadratic form

    z0 = xs^T A xs / (xs^T xs),       xs = tanh(x * scale)

with A a 10x10 real symmetric matrix depending only on theta.  Using the
eigendecomposition A = V diag(lam) V^T (V orthogonal):

    g  = V^T xs
    q  = sum(lam * g^2),   s = sum(g^2)
    out_j = (q*w_j + b_j*s) / s

The device kernel streams x in a feature-on-partition packed layout
(8 row-chunks of 10 features stacked on 80 partitions), does one ACT tanh,
two small PE matmuls (the V matvec and the lam/ones reductions) and a few
DVE ops.  Pure data-parallel across 8 NeuronCores.
"""

import numpy as np

N_QUBITS = 10
N_LAYERS = 4
N_INPUTS = 10
DIM = 2**N_QUBITS

BATCH = 32768
NCORES = 8
ROWS = BATCH // NCORES          # 4096 rows per core
C = 8                           # row-chunks stacked on partitions
NCOL = ROWS // C                # 512 columns (rows per chunk)
P = C * N_INPUTS                # 80 partitions used
W = 256                         # column tile width
T = NCOL // W                   # column tiles per core

_PROG_CACHE: dict = {}


def _compute_A(theta: np.ndarray) -> np.ndarray:
    """Collapse the circuit: A[i,j] s.t. z0 = e^T A e for the embedded state."""
    th = theta.astype(np.float64).reshape(N_LAYERS, N_QUBITS, 3)
    a, b, c = th[..., 0], th[..., 1], th[..., 2]
    cb, sb = np.cos(b / 2), np.sin(b / 2)
    e = lambda t: np.exp(1j * t)
    u00 = e(-(a + c) / 2) * cb
    u01 = -1j * e((a - c) / 2) * sb
    u10 = -1j * e(-(a - c) / 2) * sb
    u11 = e((a + c) / 2) * cb
    U = np.stack([np.stack([u00, u01], -1), np.stack([u10, u11], -1)], -2)

    M = np.zeros((DIM, N_INPUTS), np.complex128)
    for i in range(N_INPUTS):
        M[i, i] = 1.0
    for l in range(N_LAYERS):
        for q in range(N_QUBITS):
            p = M.reshape(2**q, 2, -1, N_INPUTS)
            M = np.einsum("ab,qbri->qari", U[l, q], p).reshape(DIM, N_INPUTS)
        for q in range(N_QUBITS - 1):
            p = M.reshape(2**q, 2, 2, -1, N_INPUTS).copy()
            p[:, 1] = p[:, 1, ::-1]
            M = p.reshape(DIM, N_INPUTS)
    signs = np.concatenate([np.ones(DIM // 2), -np.ones(DIM // 2)])
    return np.real(M.conj().T @ (signs[:, None] * M))


def _build_program():
    import concourse.bacc as bacc
    import concourse.mybir as mybir
    import concourse.tile as tile
    from contextlib import ExitStack

    f32 = mybir.dt.float32
    f32r = mybir.dt.float32r

    nc = bacc.Bacc(trn_type="TRN2", target_bir_lowering=False, debug=False)
    x_d = nc.dram_tensor("xp", [P, NCOL], f32, kind="ExternalInput").ap()
    sc_d = nc.dram_tensor("scale_p", [P, 1], f32, kind="ExternalInput").ap()
    v_d = nc.dram_tensor("bdv", [P, P], f32r, kind="ExternalInput").ap()
    r_d = nc.dram_tensor("red", [P, 96], f32r, kind="ExternalInput").ap()
    out_d = nc.dram_tensor("out", [ROWS, 2], f32, kind="ExternalOutput").ap()
    out_r = out_d.rearrange("(c j) o -> c (j o)", c=C)  # [C, 2*NCOL]

    with ExitStack() as ctx:
        tc = ctx.enter_context(tile.TileContext(nc))
        const = ctx.enter_context(tc.tile_pool(name="const", bufs=1))
        xin = ctx.enter_context(tc.tile_pool(name="xin", bufs=3))
        work = ctx.enter_context(tc.tile_pool(name="work", bufs=2))
        outp = ctx.enter_context(tc.tile_pool(name="outp", bufs=2))
        gps = ctx.enter_context(tc.tile_pool(name="gps", bufs=2, space="PSUM"))
        qsps = ctx.enter_context(tc.tile_pool(name="qsps", bufs=2, space="PSUM"))

        sc_t = const.tile([P, 1], f32)
        nc.sync.dma_start(sc_t[:], sc_d)
        v_t = const.tile([P, P], f32r)
        nc.sync.dma_start(v_t[:], v_d)
        r_t = const.tile([P, 96], f32r)
        nc.sync.dma_start(r_t[:], r_d)

        for t in range(T):
            xt = xin.tile([P, W], f32, tag="xt")
            nc.sync.dma_start(xt[:], x_d[:, t * W : (t + 1) * W])
            xs = work.tile([P, W], f32r, tag="xs")
            nc.scalar.activation(
                xs[:], xt[:], mybir.ActivationFunctionType.Tanh, scale=sc_t[:, 0:1]
            )
            g = gps.tile([P, W], f32, tag="g")
            nc.tensor.matmul(g[:], v_t[:], xs[:], start=True, stop=True)
            gs = work.tile([P, W], f32, tag="gs")
            nc.scalar.copy(gs[:], g[:])
            h = work.tile([P, W], f32r, tag="h")
            nc.vector.tensor_mul(h[:], gs[:], gs[:])
            qs = qsps.tile([96, W], f32, tag="qs")
            nc.tensor.matmul(qs[:], r_t[:], h[:], start=True, stop=True)
            rs = work.tile([C, W], f32, tag="rs")
            nc.vector.reciprocal(rs[:], qs[64 : 64 + C, :])
            o = outp.tile([C, 2 * W], f32, tag="o")
            nc.vector.tensor_mul(o[:, 0 : 2 * W : 2], qs[0:C, :], rs[:])
            nc.vector.tensor_mul(o[:, 1 : 2 * W : 2], qs[32 : 32 + C, :], rs[:])
            nc.sync.dma_start(out_r[:, t * 2 * W : (t + 1) * 2 * W], o[:])
    nc.compile()
    return nc


def _get_program():
    if "nc" not in _PROG_CACHE:
        _PROG_CACHE["nc"] = _build_program()
    return _PROG_CACHE["nc"]


def _host_constants(scale, theta, out_w, out_b):
    A = _compute_A(np.asarray(theta))
    lam, V = np.linalg.eigh(A)
    w = np.asarray(out_w, np.float64)[:, 0]
    b = np.asarray(out_b, np.float64)

    scale_p = np.tile(np.asarray(scale, np.float32), C)[:, None].copy()  # [P,1]
    bdv = np.kron(np.eye(C), V).astype(np.float32)                        # [P,P]
    red = np.zeros((P, 96), np.float64)
    for c in range(C):
        rows = slice(c * N_INPUTS, (c + 1) * N_INPUTS)
        red[rows, c] = lam * w[0] + b[0]
        red[rows, 32 + c] = lam * w[1] + b[1]
        red[rows, 64 + c] = 1.0
    return scale_p, bdv, red.astype(np.float32)


def kernel(x, scale, theta, out_w, out_b, _trace=False):
    from concourse.bass_utils import run_bass_kernel_spmd

    x = np.ascontiguousarray(np.asarray(x, np.float32))
    scale_p, bdv, red = _host_constants(scale, theta, out_w, out_b)

    in_maps = []
    for k in range(NCORES):
        xc = x[k * ROWS : (k + 1) * ROWS]
        xp = np.ascontiguousarray(
            xc.reshape(C, NCOL, N_INPUTS).transpose(0, 2, 1).reshape(P, NCOL)
        )
        in_maps.append({"xp": xp, "scale_p": scale_p, "bdv": bdv, "red": red})

    nc = _get_program()
    res = run_bass_kernel_spmd(
        nc, in_maps, core_ids=list(range(NCORES)), trace=_trace
    )
    out = np.concatenate([res.results[k]["out"] for k in range(NCORES)], axis=0)
    if _trace:
        return out, res
    return out


# revision 12
# speedup vs baseline: 1.2379x; 1.2379x over previous
"""Trainium2 kernel for nn_PennyLaneQuantumClassifier.

Math: the quantum circuit is linear in the state vector, and the state is
amplitude-encoded from only N_INPUTS=10 real amplitudes.  Hence the PauliZ
expectation collapses to a quadratic form

    z0 = xs^T A xs / (xs^T xs),       xs = tanh(x * scale)

with A a 10x10 real symmetric matrix depending only on theta.  Using the
eigendecomposition A = V diag(lam) V^T (V orthogonal):

    g  = V^T xs
    t_j = sum((lam*w_j + b_j) * g^2)   (j = 0, 1)
    s   = sum(g^2)                      (= |xs|^2, V orthogonal)
    out_j = t_j / s

The device kernel streams x in a feature-on-partition packed layout
(8 row-chunks of 10 features stacked on 80 partitions): one ACT tanh,
one PE matvec (block-diag V), one ACT square, one PE reduction matmul
(t0/t1/s at 32-aligned partition groups), a reciprocal and two
interleaving multiplies.  Pure data-parallel across 8 NeuronCores.
"""

import numpy as np

N_QUBITS = 10
N_LAYERS = 4
N_INPUTS = 10
DIM = 2**N_QUBITS

BATCH = 32768
NCORES = 8
ROWS = BATCH // NCORES          # 4096 rows per core
C = 8                           # row-chunks stacked on partitions
NCOL = ROWS // C                # 512 columns (rows per chunk)
P = C * N_INPUTS                # 80 partitions used
NCONST = 1 + P + 96             # scale | bdv | red columns

T = 2                           # column tiles per core
RECIP_ENGINE = "act"            # "act" or "dve"

_PROG_CACHE: dict = {}


def _compute_A(theta: np.ndarray) -> np.ndarray:
    """Collapse the circuit: A[i,j] s.t. z0 = e^T A e for the embedded state."""
    th = theta.astype(np.float64).reshape(N_LAYERS, N_QUBITS, 3)
    a, b, c = th[..., 0], th[..., 1], th[..., 2]
    cb, sb = np.cos(b / 2), np.sin(b / 2)
    e = lambda t: np.exp(1j * t)
    u00 = e(-(a + c) / 2) * cb
    u01 = -1j * e((a - c) / 2) * sb
    u10 = -1j * e(-(a - c) / 2) * sb
    u11 = e((a + c) / 2) * cb
    U = np.stack([np.stack([u00, u01], -1), np.stack([u10, u11], -1)], -2)

    M = np.zeros((DIM, N_INPUTS), np.complex128)
    for i in range(N_INPUTS):
        M[i, i] = 1.0
    for l in range(N_LAYERS):
        for q in range(N_QUBITS):
            p = M.reshape(2**q, 2, -1, N_INPUTS)
            M = np.einsum("ab,qbri->qari", U[l, q], p).reshape(DIM, N_INPUTS)
        for q in range(N_QUBITS - 1):
            p = M.reshape(2**q, 2, 2, -1, N_INPUTS).copy()
            p[:, 1] = p[:, 1, ::-1]
            M = p.reshape(DIM, N_INPUTS)
    signs = np.concatenate([np.ones(DIM // 2), -np.ones(DIM // 2)])
    return np.real(M.conj().T @ (signs[:, None] * M))


def _act_reciprocal(nc, mybir, out, in_):
    """ACT Reciprocal without the bass accuracy guard (validated on HW)."""
    eng = nc.scalar
    return eng.add_instruction(
        mybir.InstActivation(
            name=nc.get_next_instruction_name(),
            func=mybir.ActivationFunctionType.Reciprocal,
            ins=[
                eng.lower_ap(in_),
                mybir.ImmediateValue(dtype=mybir.dt.float32, value=0.0),
                mybir.ImmediateValue(dtype=mybir.dt.float32, value=1.0),
                mybir.ImmediateValue(dtype=mybir.dt.float32, value=0.0),
            ],
            outs=[eng.lower_ap(out)],
        )
    )


def _build_program():
    import concourse.bacc as bacc
    import concourse.mybir as mybir
    import concourse.tile as tile
    from contextlib import ExitStack

    f32 = mybir.dt.float32
    f32r = mybir.dt.float32r
    W = NCOL // T

    nc = bacc.Bacc(trn_type="TRN2", target_bir_lowering=False, debug=False)
    x_d = nc.dram_tensor("xp", [P, NCOL], f32, kind="ExternalInput").ap()
    c_d = nc.dram_tensor("consts", [P, NCONST], f32r, kind="ExternalInput").ap()
    out_d = nc.dram_tensor("out", [ROWS, 2], f32, kind="ExternalOutput").ap()
    out_r = out_d.rearrange("(c j) o -> c (j o)", c=C)  # [C, 2*NCOL]

    with ExitStack() as ctx:
        tc = ctx.enter_context(tile.TileContext(nc))
        const = ctx.enter_context(tc.tile_pool(name="const", bufs=1))
        xin = ctx.enter_context(tc.tile_pool(name="xin", bufs=T + 1))
        work = ctx.enter_context(tc.tile_pool(name="work", bufs=2))
        outp = ctx.enter_context(tc.tile_pool(name="outp", bufs=2))
        gps = ctx.enter_context(tc.tile_pool(name="gps", bufs=2, space="PSUM"))
        qsps = ctx.enter_context(tc.tile_pool(name="qsps", bufs=2, space="PSUM"))

        # warm the tanh ACT table while DMAs are in flight
        warm = const.tile([1, 2], f32)
        nc.gpsimd.memset(warm[:, 0:1], 0.0)
        nc.scalar.activation(
            warm[:, 1:2], warm[:, 0:1], mybir.ActivationFunctionType.Tanh
        )

        # x tiles first (they gate the tanh), then the fused consts
        xts = []
        for t in range(T):
            xt = xin.tile([P, W], f32, tag="xt")
            nc.sync.dma_start(xt[:], x_d[:, t * W : (t + 1) * W])
            xts.append(xt)
        c_t = const.tile([P, NCONST], f32r)
        nc.gpsimd.dma_start(c_t[:], c_d)
        sc_ap = c_t[:, 0:1].bitcast(f32)
        v_ap = c_t[:, 1 : 1 + P]
        r_ap = c_t[:, 1 + P : 1 + P + 96]

        for t in range(T):
            xs = work.tile([P, W], f32r, tag="xs")
            nc.scalar.activation(
                xs[:], xts[t][:], mybir.ActivationFunctionType.Tanh, scale=sc_ap
            )
            g = gps.tile([P, W], f32, tag="g")
            nc.tensor.matmul(g[:], v_ap, xs[:], start=True, stop=True)
            h = work.tile([P, W], f32r, tag="h")
            nc.scalar.activation(
                h[:], g[:], mybir.ActivationFunctionType.Square
            )
            qs = qsps.tile([96, W], f32, tag="qs")
            nc.tensor.matmul(qs[:], r_ap, h[:], start=True, stop=True)
            rs = work.tile([C, W], f32, tag="rs")
            if RECIP_ENGINE == "act":
                _act_reciprocal(nc, mybir, rs[:], qs[64 : 64 + C, :])
            else:
                nc.vector.reciprocal(rs[:], qs[64 : 64 + C, :])
            o = outp.tile([C, 2 * W], f32, tag="o")
            nc.vector.tensor_mul(o[:, 0 : 2 * W : 2], qs[0:C, :], rs[:])
            nc.vector.tensor_mul(o[:, 1 : 2 * W : 2], qs[32 : 32 + C, :], rs[:])
            nc.sync.dma_start(out_r[:, t * 2 * W : (t + 1) * 2 * W], o[:])
    nc.compile()
    return nc


def _get_program():
    if "nc" not in _PROG_CACHE:
        _PROG_CACHE["nc"] = _build_program()
    return _PROG_CACHE["nc"]


def _host_constants(scale, theta, out_w, out_b):
    A = _compute_A(np.asarray(theta))
    lam, V = np.linalg.eigh(A)
    w = np.asarray(out_w, np.float64)[:, 0]
    b = np.asarray(out_b, np.float64)

    consts = np.zeros((P, NCONST), np.float64)
    consts[:, 0] = np.tile(np.asarray(scale, np.float64), C)
    consts[:, 1 : 1 + P] = np.kron(np.eye(C), V)
    red = np.zeros((P, 96), np.float64)
    for c in range(C):
        rows = slice(c * N_INPUTS, (c + 1) * N_INPUTS)
        red[rows, c] = lam * w[0] + b[0]
        red[rows, 32 + c] = lam * w[1] + b[1]
        red[rows, 64 + c] = 1.0
    consts[:, 1 + P : 1 + P + 96] = red
    return np.ascontiguousarray(consts.astype(np.float32))


def kernel(x, scale, theta, out_w, out_b, _trace=False):
    from concourse.bass_utils import run_bass_kernel_spmd

    x = np.ascontiguousarray(np.asarray(x, np.float32))
    consts = _host_constants(scale, theta, out_w, out_b)

    in_maps = []
    for k in range(NCORES):
        xc = x[k * ROWS : (k + 1) * ROWS]
        xp = np.ascontiguousarray(
            xc.reshape(C, NCOL, N_INPUTS).transpose(0, 2, 1).reshape(P, NCOL)
        )
        in_maps.append({"xp": xp, "consts": consts})

    nc = _get_program()
    res = run_bass_kernel_spmd(
        nc, in_maps, core_ids=list(range(NCORES)), trace=_trace
    )
    out = np.concatenate([res.results[k]["out"] for k in range(NCORES)], axis=0)
    if _trace:
        return out, res
    return out


# revision 13
# speedup vs baseline: 1.2563x; 1.0149x over previous
"""Trainium2 kernel for nn_PennyLaneQuantumClassifier.

Math: the quantum circuit is linear in the state vector, and the state is
amplitude-encoded from only N_INPUTS=10 real amplitudes.  Hence the PauliZ
expectation collapses to a quadratic form

    z0 = xs^T A xs / (xs^T xs),       xs = tanh(x * scale)

with A a 10x10 real symmetric matrix depending only on theta.  Using the
eigendecomposition A = V diag(lam) V^T (V orthogonal):

    g  = V^T xs
    t_j = sum((lam*w_j + b_j) * g^2)   (j = 0, 1)
    s   = sum(g^2)                      (= |xs|^2, V orthogonal)
    out_j = t_j / s

The device kernel streams x in a feature-on-partition packed layout
(8 row-chunks of 10 features stacked on 80 partitions): one ACT tanh,
one PE matvec (block-diag V), one ACT square, one PE reduction matmul
(t0/t1/s at 32-aligned partition groups), a reciprocal and two
interleaving multiplies.  Pure data-parallel across 8 NeuronCores.
"""

import numpy as np

N_QUBITS = 10
N_LAYERS = 4
N_INPUTS = 10
DIM = 2**N_QUBITS

BATCH = 32768
NCORES = 8
ROWS = BATCH // NCORES          # 4096 rows per core
C = 8                           # row-chunks stacked on partitions
NCOL = ROWS // C                # 512 columns (rows per chunk)
P = C * N_INPUTS                # 80 partitions used
NCONST = 1 + P + 96             # scale | bdv | red columns

T = 2                           # column tiles per core
RECIP_ENGINE = "act"            # "act" or "dve"

_PROG_CACHE: dict = {}


def _compute_A(theta: np.ndarray) -> np.ndarray:
    """Collapse the circuit: A[i,j] s.t. z0 = e^T A e for the embedded state."""
    th = theta.astype(np.float64).reshape(N_LAYERS, N_QUBITS, 3)
    a, b, c = th[..., 0], th[..., 1], th[..., 2]
    cb, sb = np.cos(b / 2), np.sin(b / 2)
    e = lambda t: np.exp(1j * t)
    u00 = e(-(a + c) / 2) * cb
    u01 = -1j * e((a - c) / 2) * sb
    u10 = -1j * e(-(a - c) / 2) * sb
    u11 = e((a + c) / 2) * cb
    U = np.stack([np.stack([u00, u01], -1), np.stack([u10, u11], -1)], -2)

    M = np.zeros((DIM, N_INPUTS), np.complex128)
    for i in range(N_INPUTS):
        M[i, i] = 1.0
    for l in range(N_LAYERS):
        for q in range(N_QUBITS):
            p = M.reshape(2**q, 2, -1, N_INPUTS)
            M = np.einsum("ab,qbri->qari", U[l, q], p).reshape(DIM, N_INPUTS)
        for q in range(N_QUBITS - 1):
            p = M.reshape(2**q, 2, 2, -1, N_INPUTS).copy()
            p[:, 1] = p[:, 1, ::-1]
            M = p.reshape(DIM, N_INPUTS)
    signs = np.concatenate([np.ones(DIM // 2), -np.ones(DIM // 2)])
    return np.real(M.conj().T @ (signs[:, None] * M))


def _act_reciprocal(nc, mybir, out, in_):
    """ACT Reciprocal without the bass accuracy guard (validated on HW)."""
    eng = nc.scalar
    return eng.add_instruction(
        mybir.InstActivation(
            name=nc.get_next_instruction_name(),
            func=mybir.ActivationFunctionType.Reciprocal,
            ins=[
                eng.lower_ap(in_),
                mybir.ImmediateValue(dtype=mybir.dt.float32, value=0.0),
                mybir.ImmediateValue(dtype=mybir.dt.float32, value=1.0),
                mybir.ImmediateValue(dtype=mybir.dt.float32, value=0.0),
            ],
            outs=[eng.lower_ap(out)],
        )
    )


def _build_program():
    import concourse.bacc as bacc
    import concourse.mybir as mybir
    import concourse.tile as tile
    from contextlib import ExitStack

    f32 = mybir.dt.float32
    f32r = mybir.dt.float32r
    W = NCOL // T

    nc = bacc.Bacc(trn_type="TRN2", target_bir_lowering=False, debug=False)
    x_d = nc.dram_tensor("xp", [P, NCOL], f32, kind="ExternalInput").ap()
    c_d = nc.dram_tensor("consts", [P, NCONST], f32r, kind="ExternalInput").ap()
    out_d = nc.dram_tensor("out", [ROWS, 2], f32, kind="ExternalOutput").ap()
    out_r = out_d.rearrange("(c j) o -> c (j o)", c=C)  # [C, 2*NCOL]

    with ExitStack() as ctx:
        tc = ctx.enter_context(tile.TileContext(nc))
        const = ctx.enter_context(tc.tile_pool(name="const", bufs=1))
        xin = ctx.enter_context(tc.tile_pool(name="xin", bufs=T + 1))
        work = ctx.enter_context(tc.tile_pool(name="work", bufs=2))
        outp = ctx.enter_context(tc.tile_pool(name="outp", bufs=2))
        gps = ctx.enter_context(tc.tile_pool(name="gps", bufs=2, space="PSUM"))
        qsps = ctx.enter_context(tc.tile_pool(name="qsps", bufs=2, space="PSUM"))

        # warm the tanh ACT table while DMAs are in flight
        warm = const.tile([1, 1], f32)
        zero_ap = nc.const_aps.aps[(f32, 0.0)][0:1, 0:1]
        nc.scalar.activation(warm[:], zero_ap, mybir.ActivationFunctionType.Tanh)

        # x first (it gates the tanh), then the fused consts, on the HW queue
        xt = xin.tile([P, NCOL], f32, tag="xt")
        nc.sync.dma_start(xt[:], x_d)
        c_t = const.tile([P, NCONST], f32r)
        nc.sync.dma_start(c_t[:], c_d)
        sc_ap = c_t[:, 0:1].bitcast(f32)
        v_ap = c_t[:, 1 : 1 + P]
        r_ap = c_t[:, 1 + P : 1 + P + 96]

        # stage tiles per column-tile; recip0 is emitted before sq1 so the
        # reciprocal table load overlaps tile-1 matmuls instead of gating
        xss, gs_, hs, qss, rss, os_ = [], [], [], [], [], []
        for t in range(T):
            sl = slice(t * (NCOL // T), (t + 1) * (NCOL // T))
            xs = work.tile([P, W], f32r, tag="xs")
            nc.scalar.activation(
                xs[:], xt[:, sl], mybir.ActivationFunctionType.Tanh, scale=sc_ap
            )
            xss.append(xs)
        for t in range(T):
            g = gps.tile([P, W], f32, tag="g")
            nc.tensor.matmul(g[:], v_ap, xss[t][:], start=True, stop=True)
            gs_.append(g)
        for t in range(T):
            h = work.tile([P, W], f32r, tag="h")
            nc.scalar.activation(h[:], gs_[t][:], mybir.ActivationFunctionType.Square)
            hs.append(h)
            qs = qsps.tile([96, W], f32, tag="qs")
            nc.tensor.matmul(qs[:], r_ap, h[:], start=True, stop=True)
            qss.append(qs)
            rs = work.tile([C, W], f32, tag="rs")
            if RECIP_ENGINE == "act":
                _act_reciprocal(nc, mybir, rs[:], qs[64 : 64 + C, :])
            else:
                nc.vector.reciprocal(rs[:], qs[64 : 64 + C, :])
            rss.append(rs)
        for t in range(T):
            qs, rs = qss[t], rss[t]
            o = outp.tile([C, 2 * W], f32, tag="o")
            nc.vector.tensor_mul(o[:, 0 : 2 * W : 2], qs[0:C, :], rs[:])
            nc.vector.tensor_mul(o[:, 1 : 2 * W : 2], qs[32 : 32 + C, :], rs[:])
            nc.sync.dma_start(out_r[:, t * 2 * W : (t + 1) * 2 * W], o[:])
    nc.compile()
    return nc


def _get_program():
    if "nc" not in _PROG_CACHE:
        _PROG_CACHE["nc"] = _build_program()
    return _PROG_CACHE["nc"]


def _host_constants(scale, theta, out_w, out_b):
    A = _compute_A(np.asarray(theta))
    lam, V = np.linalg.eigh(A)
    w = np.asarray(out_w, np.float64)[:, 0]
    b = np.asarray(out_b, np.float64)

    consts = np.zeros((P, NCONST), np.float64)
    consts[:, 0] = np.tile(np.asarray(scale, np.float64), C)
    consts[:, 1 : 1 + P] = np.kron(np.eye(C), V)
    red = np.zeros((P, 96), np.float64)
    for c in range(C):
        rows = slice(c * N_INPUTS, (c + 1) * N_INPUTS)
        red[rows, c] = lam * w[0] + b[0]
        red[rows, 32 + c] = lam * w[1] + b[1]
        red[rows, 64 + c] = 1.0
    consts[:, 1 + P : 1 + P + 96] = red
    return np.ascontiguousarray(consts.astype(np.float32))


def kernel(x, scale, theta, out_w, out_b, _trace=False):
    from concourse.bass_utils import run_bass_kernel_spmd

    x = np.ascontiguousarray(np.asarray(x, np.float32))
    consts = _host_constants(scale, theta, out_w, out_b)

    in_maps = []
    for k in range(NCORES):
        xc = x[k * ROWS : (k + 1) * ROWS]
        xp = np.ascontiguousarray(
            xc.reshape(C, NCOL, N_INPUTS).transpose(0, 2, 1).reshape(P, NCOL)
        )
        in_maps.append({"xp": xp, "consts": consts})

    nc = _get_program()
    res = run_bass_kernel_spmd(
        nc, in_maps, core_ids=list(range(NCORES)), trace=_trace
    )
    out = np.concatenate([res.results[k]["out"] for k in range(NCORES)], axis=0)
    if _trace:
        return out, res
    return out


# revision 16
# speedup vs baseline: 1.3293x; 1.0581x over previous
"""Trainium2 kernel for nn_PennyLaneQuantumClassifier.

Math: the quantum circuit is linear in the state vector, and the state is
amplitude-encoded from only N_INPUTS=10 real amplitudes.  Hence the PauliZ
expectation collapses to a quadratic form

    z0 = xs^T A xs / (xs^T xs),       xs = tanh(x * scale)

with A a 10x10 real symmetric matrix depending only on theta.  Using the
eigendecomposition A = V diag(lam) V^T (V orthogonal):

    g  = V^T xs
    t_j = sum((lam*w_j + b_j) * g^2)   (j = 0, 1)
    s   = sum(g^2)                      (= |xs|^2, V orthogonal)
    out_j = t_j / s

The device kernel streams x in a feature-on-partition packed layout
(8 row-chunks of 10 features stacked on 80 partitions): one ACT tanh,
one PE matvec (block-diag V), one ACT square, one PE reduction matmul
(t0/t1/s at 32-aligned partition groups), a reciprocal and two
interleaving multiplies.  Pure data-parallel across 8 NeuronCores.
"""

import numpy as np

N_QUBITS = 10
N_LAYERS = 4
N_INPUTS = 10
DIM = 2**N_QUBITS

BATCH = 32768
NCORES = 8
ROWS = BATCH // NCORES          # 4096 rows per core
C = 8                           # row-chunks stacked on partitions
NCOL = ROWS // C                # 512 columns (rows per chunk)
P = C * N_INPUTS                # 80 partitions used
NCONST = 1 + P + 96             # scale | bdv | red columns

T = 2                           # column tiles per core
RECIP_ENGINE = "act"            # "act" or "dve"

_PROG_CACHE: dict = {}


def _compute_A(theta: np.ndarray) -> np.ndarray:
    """Collapse the circuit: A[i,j] s.t. z0 = e^T A e for the embedded state."""
    th = theta.astype(np.float64).reshape(N_LAYERS, N_QUBITS, 3)
    a, b, c = th[..., 0], th[..., 1], th[..., 2]
    cb, sb = np.cos(b / 2), np.sin(b / 2)
    e = lambda t: np.exp(1j * t)
    u00 = e(-(a + c) / 2) * cb
    u01 = -1j * e((a - c) / 2) * sb
    u10 = -1j * e(-(a - c) / 2) * sb
    u11 = e((a + c) / 2) * cb
    U = np.stack([np.stack([u00, u01], -1), np.stack([u10, u11], -1)], -2)

    M = np.zeros((DIM, N_INPUTS), np.complex128)
    for i in range(N_INPUTS):
        M[i, i] = 1.0
    for l in range(N_LAYERS):
        for q in range(N_QUBITS):
            p = M.reshape(2**q, 2, -1, N_INPUTS)
            M = np.einsum("ab,qbri->qari", U[l, q], p).reshape(DIM, N_INPUTS)
        for q in range(N_QUBITS - 1):
            p = M.reshape(2**q, 2, 2, -1, N_INPUTS).copy()
            p[:, 1] = p[:, 1, ::-1]
            M = p.reshape(DIM, N_INPUTS)
    signs = np.concatenate([np.ones(DIM // 2), -np.ones(DIM // 2)])
    return np.real(M.conj().T @ (signs[:, None] * M))


def _act_reciprocal(nc, mybir, out, in_):
    """ACT Reciprocal without the bass accuracy guard (validated on HW)."""
    eng = nc.scalar
    return eng.add_instruction(
        mybir.InstActivation(
            name=nc.get_next_instruction_name(),
            func=mybir.ActivationFunctionType.Reciprocal,
            ins=[
                eng.lower_ap(in_),
                mybir.ImmediateValue(dtype=mybir.dt.float32, value=0.0),
                mybir.ImmediateValue(dtype=mybir.dt.float32, value=1.0),
                mybir.ImmediateValue(dtype=mybir.dt.float32, value=0.0),
            ],
            outs=[eng.lower_ap(out)],
        )
    )


def _build_program():
    import concourse.bacc as bacc
    import concourse.mybir as mybir
    from contextlib import ExitStack

    f32 = mybir.dt.float32
    f32r = mybir.dt.float32r
    W = NCOL // T
    Tanh = mybir.ActivationFunctionType.Tanh
    Square = mybir.ActivationFunctionType.Square

    nc = bacc.Bacc(trn_type="TRN2", target_bir_lowering=False, debug=False)
    x_d = nc.dram_tensor("xp", [P, NCOL], f32, kind="ExternalInput").ap()
    c_d = nc.dram_tensor("consts", [P, NCONST], f32r, kind="ExternalInput").ap()
    out_d = nc.dram_tensor("out", [ROWS, 2], f32, kind="ExternalOutput").ap()
    out_r = out_d.rearrange("(c j) o -> c (j o)", c=C)  # [C, 2*NCOL]

    warm = nc.alloc_sbuf_tensor("warm", [1, 1], f32).ap()
    xt = nc.alloc_sbuf_tensor("xt_raw", [P, NCOL], f32).ap()
    c_t = nc.alloc_sbuf_tensor("c_raw", [P, NCONST], f32r).ap()
    sc_ap = c_t[:, 0:1].bitcast(f32)
    v_ap = c_t[:, 1 : 1 + P]
    r_ap = c_t[:, 1 + P : 1 + P + 96]
    xs = [nc.alloc_sbuf_tensor(f"xs{t}", [P, W], f32r).ap() for t in range(T)]
    h = [nc.alloc_sbuf_tensor(f"h{t}", [P, W], f32r).ap() for t in range(T)]
    ss = [nc.alloc_sbuf_tensor(f"ss{t}", [C, W], f32).ap() for t in range(T)]
    rs = [nc.alloc_sbuf_tensor(f"rs{t}", [C, W], f32).ap() for t in range(T)]
    o = [nc.alloc_sbuf_tensor(f"o{t}", [C, 2 * W], f32).ap() for t in range(T)]

    in_sem = nc.alloc_semaphore("in_dma")
    out_sem = nc.alloc_semaphore("out_dma")
    act_sem = nc.alloc_semaphore("act")
    pe_sem = nc.alloc_semaphore("pe")
    dve_sem = nc.alloc_semaphore("dve")

    with ExitStack() as ctx:
        g = [
            ctx.enter_context(nc.psum_tensor(f"g{t}", [P, W], f32)).ap()
            for t in range(T)
        ]
        qs = [
            ctx.enter_context(nc.psum_tensor(f"qs{t}", [96, W], f32)).ap()
            for t in range(T)
        ]

        # SP: input DMA triggers first, then gated output DMAs
        nc.sync.dma_start(xt, x_d).then_inc(in_sem, 16)
        nc.sync.dma_start(c_t, c_d).then_inc(in_sem, 16)
        for t in range(T):
            nc.sync.dma_start(
                out_r[:, t * 2 * W : (t + 1) * 2 * W], o[t]
            )._wait_ge(dve_sem, 3 * (t + 1)).then_inc(out_sem, 16)
        nc.sync.wait_ge(out_sem, 32)

        # ACT: warm table, tanh, square, s-copy.  act_sem counts from memzero.
        nc.scalar.memzero(warm).then_inc(act_sem, 1)
        nc.scalar.activation(warm, warm, Tanh).then_inc(act_sem, 1)
        for t in range(T):
            nc.scalar.activation(
                xs[t], xt[:, t * W : (t + 1) * W], Tanh, scale=sc_ap
            )._wait_ge(in_sem, 32).then_inc(act_sem, 1)  # act 3, 4
        for t in range(T):
            nc.scalar.activation(h[t], g[t], Square)._wait_ge(
                pe_sem, t + 1
            ).then_inc(act_sem, 1)  # act 5, 6
        for t in range(T):
            nc.scalar.copy(ss[t], qs[t][64 : 64 + C, :])._wait_ge(
                pe_sem, 3 + t
            ).then_inc(act_sem, 1)  # act 7, 8

        # PE: two matvecs, two reductions
        for t in range(T):
            nc.tensor.matmul(
                g[t], v_ap, xs[t], start=True, stop=True
            )._wait_ge(act_sem, 3 + t).then_inc(pe_sem, 1)  # pe 1, 2
        for t in range(T):
            nc.tensor.matmul(
                qs[t], r_ap, h[t], start=True, stop=True
            )._wait_ge(act_sem, 5 + t).then_inc(pe_sem, 1)  # pe 3, 4

        # DVE: reciprocal + interleaving output muls
        for t in range(T):
            nc.vector.reciprocal_approx_fast(out=rs[t], in_=ss[t])._wait_ge(
                act_sem, 7 + t
            ).then_inc(dve_sem, 1)  # dve 1, 4
            nc.vector.tensor_mul(
                o[t][:, 0 : 2 * W : 2], qs[t][0:C, :], rs[t]
            ).then_inc(dve_sem, 1)  # dve 2, 5
            nc.vector.tensor_mul(
                o[t][:, 1 : 2 * W : 2], qs[t][32 : 32 + C, :], rs[t]
            ).then_inc(dve_sem, 1)  # dve 3, 6

        nc.compile()
    return nc


def _get_program():
    if "nc" not in _PROG_CACHE:
        _PROG_CACHE["nc"] = _build_program()
    return _PROG_CACHE["nc"]


def _host_constants(scale, theta, out_w, out_b):
    A = _compute_A(np.asarray(theta))
    lam, V = np.linalg.eigh(A)
    w = np.asarray(out_w, np.float64)[:, 0]
    b = np.asarray(out_b, np.float64)

    consts = np.zeros((P, NCONST), np.float64)
    consts[:, 0] = np.tile(np.asarray(scale, np.float64), C)
    consts[:, 1 : 1 + P] = np.kron(np.eye(C), V)
    red = np.zeros((P, 96), np.float64)
    for c in range(C):
        rows = slice(c * N_INPUTS, (c + 1) * N_INPUTS)
        red[rows, c] = lam * w[0] + b[0]
        red[rows, 32 + c] = lam * w[1] + b[1]
        red[rows, 64 + c] = 1.0
    consts[:, 1 + P : 1 + P + 96] = red
    return np.ascontiguousarray(consts.astype(np.float32))


def kernel(x, scale, theta, out_w, out_b, _trace=False):
    from concourse.bass_utils import run_bass_kernel_spmd

    x = np.ascontiguousarray(np.asarray(x, np.float32))
    consts = _host_constants(scale, theta, out_w, out_b)

    in_maps = []
    for k in range(NCORES):
        xc = x[k * ROWS : (k + 1) * ROWS]
        xp = np.ascontiguousarray(
            xc.reshape(C, NCOL, N_INPUTS).transpose(0, 2, 1).reshape(P, NCOL)
        )
        in_maps.append({"xp": xp, "consts": consts})

    nc = _get_program()
    res = run_bass_kernel_spmd(
        nc, in_maps, core_ids=list(range(NCORES)), trace=_trace
    )
    out = np.concatenate([res.results[k]["out"] for k in range(NCORES)], axis=0)
    if _trace:
        return out, res
    return out
